# revision 1
# baseline (speedup 1.0000x reference)
"""Multi-head causal attention (B=8, T=2048, C=384, H=6, Dh=64) on 8 TRN2 cores.

Sharding: data-parallel over batch — core b computes batch element b end to end
(no collectives).

Per-core kernel layout (all "T" means transposed, head-dim/channel on
partitions):
  xT   [128, 3, 2048]  bf16   c = 128*ci + p
  wq/wk[128, 3, 384]   bf16   packed Wq[h,c,d] -> [c, h*64+d]
  wv   [128, 3, 384]   bf16
  wp   [128, 3, 384]   bf16   Wp[c, e] -> [128, ci, e]
  bp   [128, 384]      f32    bias broadcast rows
  iden [6, 6]          f32    eye(6), transpose helper

Compute per core:
  QT/KT [hd, t] via matmul(lhsT=w chunk, rhs=xT)      (hd = h*64+d, 3 blocks)
  V_aug [s, 65] per (s-chunk, head), last col = 1     (stationary for PV)
  per q-block j (512 wide), head h (software-pipelined one S ahead):
    ST chunks [s=128, t<=512] = KT^T-slice @ QT-slice (K = d = 64); fringe
    chunks get an additive -1e30 causal mask matmul (idn128^T @ maskc)
    emitted FIRST in the same PSUM accumulation group, so masking costs no
    chain latency and GpSimd stays out of the exp->PV critical path
    exp (ACT, scale=Dh^-0.5) -> P bf16
    O_aug [65, d:512] += V_aug^T @ P[:, d:512]        (row 64 = softmax denom)
    denom rows staged into den6 [6, 512] per block
  block j's den6 is batch-transposed (4x [6,128] -> [128,6]) + recip'd lazily
  at the start of block j+1 so the PE never waits at block boundaries;
  out-proj of block j-1 interleaved into block j's head loop.

HAM clock-gate management: the TRN2 PE runs at 1.2 GHz until it has been
busy with zero gaps for a full 4096-cycle window (~3.4us), then 2.4 GHz.
A dependency-free "warm burst" accumulation group at the attention-phase
start trips the gate; with the mask on the PE the chain itself is gapless
enough to hold 2.4 GHz for the whole attention phase, which is then paced
by ACT exp throughput. The next head's first S chunks are prefetched into
the pipeline slots freed at each chain tail so ACT never idles at head
starts; the ACT exp table is preloaded in phase 1.
"""

import numpy as np
import ml_dtypes

import concourse.bass as bass
import concourse.tile as tile
from concourse import bacc, mybir
from concourse.bass import ts, ds

F32 = mybir.dt.float32
BF16 = mybir.dt.bfloat16
AF = mybir.ActivationFunctionType

B, T, C = 8, 2048, 384
H, DH = 6, 64
SCALE = DH ** -0.5
NCORES = 8
TJ = 512            # q-block width
NJ = T // TJ        # 4 q-blocks
SC = 128            # s-chunk
NCI = C // 128      # 3 channel chunks
NCH = TJ // SC      # s-chunks per q-block (4)


def build_kernel():
    nc = bacc.Bacc("TRN2", target_bir_lowering=False, debug=False)

    xT_d = nc.dram_tensor("xT", [128, NCI, T], BF16, kind="ExternalInput").ap()
    wq_d = nc.dram_tensor("wq", [128, NCI, C], BF16, kind="ExternalInput").ap()
    wk_d = nc.dram_tensor("wk", [128, NCI, C], BF16, kind="ExternalInput").ap()
    wv_d = nc.dram_tensor("wv", [128, NCI, C], BF16, kind="ExternalInput").ap()
    wp_d = nc.dram_tensor("wp", [128, NCI, C], BF16, kind="ExternalInput").ap()
    biasb_d = nc.dram_tensor("biasb", [128, 384], F32, kind="ExternalInput").ap()
    iden_d = nc.dram_tensor("iden", [6, 6], F32, kind="ExternalInput").ap()
    maskc_d = nc.dram_tensor("maskc", [128, 128], BF16, kind="ExternalInput").ap()
    idn128_d = nc.dram_tensor("idn128", [128, 128], BF16, kind="ExternalInput").ap()
    y_d = nc.dram_tensor("y", [T, C], F32, kind="ExternalOutput").ap()

    with tile.TileContext(nc) as tc:
        with tc.tile_pool(name="const", bufs=1) as cpool:
            xT = cpool.tile([128, NCI, T], BF16)
            wq = cpool.tile([128, NCI, C], BF16)
            wk = cpool.tile([128, NCI, C], BF16)
            wv = cpool.tile([128, NCI, C], BF16)
            wp = cpool.tile([128, NCI, C], BF16)
            biasb = cpool.tile([128, 384], F32)
            iden = cpool.tile([6, 6], F32)
            maskc = cpool.tile([128, 128], BF16)
            idn128 = cpool.tile([128, 128], BF16)
            QT = cpool.tile([128, NCI, T], BF16)
            KT = cpool.tile([128, NCI, T], BF16)
            attT = cpool.tile([128, NCI, T], BF16)
            Vt = cpool.tile([128, 16, H, 65], BF16)

            nc.sync.dma_start(wq[:], wq_d[:])
            nc.sync.dma_start(wk[:], wk_d[:])
            for ci in range(NCI):
                for th in range(2):
                    nc.sync.dma_start(xT[:, ci, ts(th, T // 2)],
                                      xT_d[:, ci, ts(th, T // 2)])
            nc.sync.dma_start(wv[:], wv_d[:])
            nc.sync.dma_start(wp[:], wp_d[:])
            nc.sync.dma_start(biasb[:], biasb_d[:])
            nc.sync.dma_start(iden[:], iden_d[:])
            nc.sync.dma_start(maskc[:], maskc_d[:])
            nc.sync.dma_start(idn128[:], idn128_d[:])
            # whole-tile memset (contiguous; strided memset fails ISA check);
            # V copies below overwrite cols 0:64, leaving col 64 == 1.0
            nc.gpsimd.memset(Vt[:], 1.0)
            # preload the ACT exp table during phase 1 so the first real exp
            # in the attention phase doesn't stall the pipeline ~1.3us
            scr = cpool.tile([1, 1], F32)
            nc.gpsimd.memset(scr[:], 0.0)
            nc.scalar.activation(scr[:], scr[:], AF.Exp, scale=1.0)

            # ---- phase 1: projections ----
            with tc.tile_pool(name="pqk", bufs=2, space="PSUM") as pqk, \
                 tc.tile_pool(name="pv", bufs=2, space="PSUM") as pvp:
                for dst, w in ((QT, wq), (KT, wk)):
                    for pi in range(NCI):
                        for tcn in range(T // 512):
                            ps = pqk.tile([128, 512], F32, tag="pqk")
                            for ci in range(NCI):
                                nc.tensor.matmul(
                                    ps[:],
                                    lhsT=w[:, ci, ts(pi, 128)],
                                    rhs=xT[:, ci, ts(tcn, 512)],
                                    start=(ci == 0), stop=(ci == NCI - 1),
                                )
                            nc.vector.tensor_copy(dst[:, pi, ts(tcn, 512)], ps[:])
                for si in range(16):
                    ps = pvp.tile([128, C], F32, tag="pv")
                    for ci in range(NCI):
                        nc.tensor.matmul(
                            ps[:],
                            lhsT=xT[:, ci, ts(si, 128)],
                            rhs=wv[:, ci, :],
                            start=(ci == 0), stop=(ci == NCI - 1),
                        )
                    nc.vector.tensor_copy(
                        Vt[:, si, :, 0:64],
                        ps[:].rearrange("p (h d) -> p h d", h=H),
                    )

            # ---- phase 2+3: attention + output projection ----
            with tc.tile_pool(name="sps", bufs=3, space="PSUM") as sps, \
                 tc.tile_pool(name="ops", bufs=2, space="PSUM") as ops, \
                 tc.tile_pool(name="dps", bufs=1, space="PSUM") as dps, \
                 tc.tile_pool(name="ups", bufs=2, space="PSUM") as ups, \
                 tc.tile_pool(name="pp", bufs=3) as pp, \
                 tc.tile_pool(name="rp", bufs=2) as rp, \
                 tc.tile_pool(name="yp", bufs=2) as yp:
                rTs = {}
                den6s = {}

                def warm_burst(n):
                    # dependency-free accumulation group (no inter-matmul
                    # semaphores -> truly gapless PE stream) to trip the HAM
                    # gate; dead write into the U ring, later overwritten
                    Ub = ups.tile([128, C], F32, tag="U")
                    for k in range(n):
                        nc.tensor.matmul(
                            Ub[:], lhsT=xT[:, 0, 0:128], rhs=xT[:, 0, 0:C],
                            start=(k == 0), stop=(k == n - 1),
                        )

                def denom_finalize(jj):
                    # batch-transpose block jj's denominators + reciprocal
                    den6 = den6s.pop(jj)
                    dT = dps.tile([128, NCH, H], F32, tag="dT")
                    for q in range(NCH):
                        nc.tensor.transpose(
                            dT[:, q, :], den6[0:H, ts(q, 128)], iden[0:H, 0:H]
                        )
                    rT = rp.tile([128, NCH * H], F32, tag="rT")
                    nc.vector.reciprocal(
                        rT[:], dT[:].rearrange("p a b -> p (a b)"))
                    rTs[jj] = rT

                def out_proj(jj, q):
                    # Y[t, e] for t-chunk (jj, q): sum over heads of
                    # (attT_h^T @ wp_h) * recip_h[t], plus bias.
                    tb = NCH * jj + q
                    rT = rTs[jj]
                    Y = yp.tile([128, C], F32, tag="Y")
                    for h in range(H):
                        po = (h % 2) * 64
                        bi = h // 2
                        U = ups.tile([128, C], F32, tag="U")
                        nc.tensor.matmul(
                            U[:],
                            lhsT=attT[po:po + 64, bi, ts(tb, 128)],
                            rhs=wp[po:po + 64, bi, :],
                            start=True, stop=True,
                        )
                        sc = rT[:, q * H + h:q * H + h + 1]
                        nc.vector.scalar_tensor_tensor(
                            out=Y[:], in0=U[:], scalar=sc,
                            in1=(biasb[:] if h == 0 else Y[:]),
                            op0=mybir.AluOpType.mult,
                            op1=mybir.AluOpType.add,
                        )
                    nc.sync.dma_start(y_d[ts(tb, 128), :], Y[:])

                for j in range(NJ):
                    den6 = rp.tile([H, TJ], F32, tag="den6")
                    den6s[j] = den6
                    nch = NCH * j + NCH   # s-chunks for this q-block

                    def s_mm(hh, i):
                        po = (hh % 2) * 64
                        bi = hh // 2
                        fringe = i >= NCH * j
                        d = SC * i - TJ * j if fringe else 0
                        S = sps.tile([128, TJ], F32, tag="S")
                        if fringe:
                            # additive causal mask on the diag window, FIRST
                            # in the group (no deps, runs early) so exp only
                            # waits on the trailing S matmul; keeps GpSimd
                            # out of the exp->PV chain entirely
                            nc.tensor.matmul(
                                S[:, d:d + 128],
                                lhsT=idn128[:],
                                rhs=maskc[:],
                                start=True, stop=False,
                            )
                        nc.tensor.matmul(
                            S[:, d:TJ],
                            lhsT=KT[po:po + 64, bi, ts(i, SC)],
                            rhs=QT[po:po + 64, bi, ds(j * TJ + d, TJ - d)],
                            start=not fringe, stop=True,
                        )
                        return S, d, fringe

                    prefetched = {}
                    for h in range(H):
                        po = (h % 2) * 64     # partition offset inside hd-block
                        bi = h // 2           # hd block index

                        O = ops.tile([65, TJ], F32, tag="O")
                        # 2-deep software pipeline; the first chunks may have
                        # been prefetched during the previous head's chain
                        # tail so the ACT engine never idles at head starts
                        pend = prefetched.pop(h, [])
                        while len(pend) < 2:
                            pend.append(s_mm(h, len(pend)))
                        if j == 0 and h == 0:
                            # initial warm burst: runs while the first exps
                            # complete, and keeps the HAM gate at 2.4 GHz
                            warm_burst(18)
                        for i in range(nch):
                            S, d, fringe = pend.pop(0)
                            P = pp.tile([128, TJ], BF16, tag="P")
                            nc.scalar.activation(P[:, d:TJ], S[:, d:TJ],
                                                 AF.Exp, scale=SCALE)
                            if i + 2 < nch:
                                pend.append(s_mm(h, i + 2))
                            elif h + 1 < H:
                                # chain tail: prefetch next head's S chunks
                                prefetched.setdefault(h + 1, []).append(
                                    s_mm(h + 1, i + 2 - nch))
                            nc.tensor.matmul(
                                O[:, d:TJ],
                                lhsT=Vt[:, i, h, :],
                                rhs=P[:, d:TJ],
                                start=(i == 0), stop=(i == nch - 1),
                            )
                        # stage unnormalized attT (bf16) and denom row
                        nc.vector.tensor_copy(
                            attT[po:po + 64, bi, ts(j, TJ)], O[0:64, :]
                        )
                        dsb = rp.tile([1, TJ], F32, tag="dsb")
                        nc.vector.tensor_copy(dsb[:], O[64:65, :])
                        # SBUF->SBUF DMA can target any partition; lands the
                        # denom row on den6 partition h for batch transposing
                        nc.sync.dma_start(den6[h:h + 1, :], dsb[:])
                        if j > 0 and h == 0:
                            # lazy: previous block's denom transposes + recip,
                            # emitted here so the PE stays fed at boundaries
                            denom_finalize(j - 1)
                        if j > 0 and h < NCH:
                            # interleave previous block's output projection
                            out_proj(j - 1, h)
                        if j < 2:
                            # shallow chains stall ~0.6us on exp latency at
                            # each head start; filler matmuls keep the PE idle
                            # fraction under the HAM cold threshold
                            warm_burst(4)
                    if j < NJ - 1:
                        # insurance re-trigger at block boundaries
                        warm_burst(24 if j < 2 else 12)
                # tail: last block's output projection
                denom_finalize(NJ - 1)
                for q in range(NCH):
                    out_proj(NJ - 1, q)
                    if q < NCH - 1:
                        warm_burst(3)

    nc.compile()
    return nc


def _prep_inputs(x, Wq, Wk, Wv, Wp, bp):
    """Host-side shard + layout prep. Returns per-core input maps."""
    bf = ml_dtypes.bfloat16
    x = np.asarray(x, dtype=np.float32)

    def pack_w(W):  # [H, C, Dh] -> [128, NCI, H*Dh]
        Whd = np.transpose(np.asarray(W, np.float32), (1, 0, 2)).reshape(C, H * DH)
        return np.ascontiguousarray(
            Whd.reshape(NCI, 128, H * DH).transpose(1, 0, 2)
        ).astype(bf)

    wq_p, wk_p, wv_p = pack_w(Wq), pack_w(Wk), pack_w(Wv)
    wp_p = np.ascontiguousarray(
        np.asarray(Wp, np.float32).reshape(NCI, 128, C).transpose(1, 0, 2)
    ).astype(bf)

    biasb = np.broadcast_to(np.asarray(bp, np.float32), (128, C)).copy()
    iden_np = np.eye(6, dtype=np.float32)
    p = np.arange(128)[:, None]
    f = np.arange(128)[None, :]
    maskc_np = np.where(f >= p, 0.0, -1e30).astype(ml_dtypes.bfloat16)
    idn128_np = np.eye(128, dtype=ml_dtypes.bfloat16)

    in_maps = []
    for b in range(B):
        xT = np.ascontiguousarray(
            x[b].T.reshape(NCI, 128, T).transpose(1, 0, 2)
        ).astype(bf)
        in_maps.append({
            "xT": xT, "wq": wq_p, "wk": wk_p, "wv": wv_p, "wp": wp_p,
            "biasb": biasb, "iden": iden_np,
            "maskc": maskc_np, "idn128": idn128_np,
        })
    return in_maps


_CACHE = {}


def kernel(x, Wq, Wk, Wv, Wp, bp):
    from concourse.bass_utils import run_bass_kernel_spmd

    if "nc" not in _CACHE:
        _CACHE["nc"] = build_kernel()
    nc = _CACHE["nc"]
    in_maps = _prep_inputs(x, Wq, Wk, Wv, Wp, bp)
    res = run_bass_kernel_spmd(nc, in_maps, list(range(NCORES)))
    out = np.stack([res.results[b]["y"] for b in range(B)], axis=0)
    return out.astype(np.float32)



# revision 15
# speedup vs baseline: 1.1759x; 1.1759x over previous
"""Multi-head causal attention (B=8, T=2048, C=384, H=6, Dh=64) on 8 TRN2 cores.

Sharding: data-parallel over batch - core b computes batch element b end to end
(no collectives).

The attention phase is ACT(exp)-throughput-paced, so the kernel is organized
to (a) halve PE time via head-pair row-group concurrency, (b) cut ACT
per-instruction overhead by batching exp over head pairs, (c) keep the DVE
lean by folding softmax normalization into the mandatory PSUM->SBUF copy.

Per-core layout (partition-major):
  xT   [128, 3, 2048]  bf16   c = 128*ci + p
  wq/wk[128, 3, 384]   bf16   packed Wq[h,c,d] -> [c, h*64+d]
  wv   [128, 3, 384]   bf16
  wp2  [64, 6, 384]    bf16   Wp[h*64+d, e] -> [d, h, e]  (all heads on
                              partitions 0:64 so the out-proj accumulation
                              group stays in ONE PE row group - concurrent
                              row-group matmuls accumulating into one PSUM
                              bank crash the hardware)
  biasb[128, 384]      f32
  mask01 [128, 128]    bf16   upper-tri (incl diag) 1.0 else 0.0
  attT2 [64, 6, 2048]  bf16   NORMALIZED attention output, [d, h, t]

Compute:
  QT/KT [hd, t] via matmul; V_aug [s, 65] per (s-chunk, head), col 64 = 1.
  Heads are processed in PAIRS (2m, 2m+1): their K=64 S matmuls live at
  partition offsets 0/64, so tile_position auto-derivation runs them
  CONCURRENTLY in opposite 64-row halves of the PE array.  One exp ACTIVATE
  covers both heads' scores ([128, 2, TJ] spanning 2 PSUM banks).  Causal
  fringe masking is a post-exp DVE multiply by a 0/1 mask.  Normalization:
  the V_aug ones-row denominators (row 64 of O) are staged to SBUF (DVE;
  reciprocal_approx_fast misreads PSUM on HW), approx-reciprocal'd, gpsimd
  partition_broadcast to 64 partitions, and the PSUM->SBUF attT copy becomes
  a tensor_mul.  Output projection: 6-matmul accumulation group (one row
  group) + one DVE bias add per t-chunk.  QKV projections are emitted as
  interleaved filler inside the first attention blocks (useful PE work in
  the gaps the ACT-paced pipeline leaves, which also keeps HAM warm).
"""

import numpy as np
import ml_dtypes

import concourse.bass as bass
import concourse.tile as tile
from concourse import bacc, mybir
from concourse.bass import ts, ds

F32 = mybir.dt.float32
BF16 = mybir.dt.bfloat16
AF = mybir.ActivationFunctionType

B, T, C = 8, 2048, 384
H, DH = 6, 64
SCALE = DH ** -0.5
NCORES = 8
TJ = 512            # q-block width
NJ = T // TJ        # 4 q-blocks
SC = 128            # s-chunk
NCI = C // 128      # 3 channel chunks
NCH = TJ // SC      # s-chunks per q-block (4)
NP = H // 2         # head pairs (3)


def build_kernel():
    nc = bacc.Bacc("TRN2", target_bir_lowering=False, debug=False)

    xT_d = nc.dram_tensor("xT", [128, NCI, T], BF16, kind="ExternalInput").ap()
    wq_d = nc.dram_tensor("wq", [128, NCI, C], BF16, kind="ExternalInput").ap()
    wk_d = nc.dram_tensor("wk", [128, NCI, C], BF16, kind="ExternalInput").ap()
    wv_d = nc.dram_tensor("wv", [128, NCI, C], BF16, kind="ExternalInput").ap()
    wp2_d = nc.dram_tensor("wp2", [64, H, C], BF16, kind="ExternalInput").ap()
    biasb_d = nc.dram_tensor("biasb", [128, 384], F32, kind="ExternalInput").ap()
    mask01_d = nc.dram_tensor("mask01", [128, 128], BF16, kind="ExternalInput").ap()
    y_d = nc.dram_tensor("y", [T, C], F32, kind="ExternalOutput").ap()

    with tile.TileContext(nc) as tc:
        with tc.tile_pool(name="const", bufs=1) as cpool:
            xT = cpool.tile([128, NCI, T], BF16)
            wq = cpool.tile([128, NCI, C], BF16)
            wk = cpool.tile([128, NCI, C], BF16)
            wv = cpool.tile([128, NCI, C], BF16)
            wp2 = cpool.tile([64, H, C], BF16)
            biasb = cpool.tile([128, 384], F32)
            mask01 = cpool.tile([128, 128], BF16)
            QT = cpool.tile([128, NCI, T], BF16)
            KT = cpool.tile([128, NCI, T], BF16)
            attT2 = cpool.tile([64, H, T], BF16)   # normalized
            Vt = cpool.tile([128, 16, H, 65], BF16)

            nc.sync.dma_start(wq[:], wq_d[:])
            nc.sync.dma_start(wk[:], wk_d[:])
            for tcn in range(T // 512):
                for ci in range(NCI):
                    nc.sync.dma_start(xT[:, ci, ts(tcn, 512)],
                                      xT_d[:, ci, ts(tcn, 512)])
            nc.sync.dma_start(wv[:], wv_d[:])
            nc.sync.dma_start(wp2[:], wp2_d[:])
            nc.sync.dma_start(biasb[:], biasb_d[:])
            nc.sync.dma_start(mask01[:], mask01_d[:])
            # whole-tile memset; V copies overwrite cols 0:64, col 64 stays 1.0
            nc.gpsimd.memset(Vt[:], 1.0)
            # preload the ACT exp table so the first attention exp is cheap
            scr = cpool.tile([1, 1], F32)
            nc.gpsimd.memset(scr[:], 0.0)
            nc.scalar.activation(scr[:], scr[:], AF.Exp, scale=1.0)

            with tc.tile_pool(name="sps", bufs=2, space="PSUM") as sps, \
                 tc.tile_pool(name="ops", bufs=2, space="PSUM") as ops, \
                 tc.tile_pool(name="pp", bufs=3) as pp, \
                 tc.tile_pool(name="rp", bufs=2) as rp, \
                 tc.tile_pool(name="rbp", bufs=2) as rbp, \
                 tc.tile_pool(name="yp", bufs=2) as yp:

                # ---------- filler work (projections, out-projection) ----------
                def qk_proj(pi, tcn):
                    ps = sps.tile([128, 2, 512], F32, tag="S")
                    for k, w in ((0, wq), (1, wk)):
                        for ci in range(NCI):
                            nc.tensor.matmul(
                                ps[:, k, :],
                                lhsT=w[:, ci, ts(pi, 128)],
                                rhs=xT[:, ci, ts(tcn, 512)],
                                start=(ci == 0), stop=(ci == NCI - 1),
                            )
                    nc.vector.tensor_copy(QT[:, pi, ts(tcn, 512)], ps[:, 0, :])
                    nc.vector.tensor_copy(KT[:, pi, ts(tcn, 512)], ps[:, 1, :])

                def v_proj(si0):
                    ps = sps.tile([128, 2, 512], F32, tag="S")
                    for k in range(2):
                        si = si0 + k
                        for ci in range(NCI):
                            nc.tensor.matmul(
                                ps[:, k, 0:C],
                                lhsT=xT[:, ci, ts(si, 128)],
                                rhs=wv[:, ci, :],
                                start=(ci == 0), stop=(ci == NCI - 1),
                            )
                        nc.vector.tensor_copy(
                            Vt[:, si, :, 0:64],
                            ps[:, k, 0:C].rearrange("p (h d) -> p h d", h=H),
                        )

                def out_proj(jj, q):
                    tb = NCH * jj + q
                    Up = sps.tile([128, 2, 512], F32, tag="S")
                    U = Up[:, 0, 0:C]
                    for h in range(H):
                        nc.tensor.matmul(
                            U,
                            lhsT=attT2[:, h, ts(tb, 128)],
                            rhs=wp2[:, h, :],
                            start=(h == 0), stop=(h == H - 1),
                        )
                    Y = yp.tile([128, C], F32, tag="Y")
                    nc.vector.tensor_add(Y[:], U, biasb[:])
                    nc.sync.dma_start(y_d[ts(tb, 128), :], Y[:])

                # ---------- attention ----------
                pair_seq = [(j, m) for j in range(NJ) for m in range(NP)]
                fillers = {
                    (0, 0): [lambda: qk_proj(1, 0)],
                    (0, 1): [lambda: qk_proj(2, 0), lambda: v_proj(4)],
                    (0, 2): [lambda: qk_proj(0, 1), lambda: v_proj(6)],
                    (1, 0): [lambda: qk_proj(1, 1), lambda: out_proj(0, 0)],
                    (1, 1): [lambda: qk_proj(2, 1), lambda: v_proj(8),
                             lambda: out_proj(0, 1)],
                    (1, 2): [lambda: qk_proj(0, 2), lambda: v_proj(10),
                             lambda: out_proj(0, 2), lambda: out_proj(0, 3)],
                    (2, 0): [lambda: qk_proj(1, 2), lambda: out_proj(1, 0)],
                    (2, 1): [lambda: qk_proj(2, 2), lambda: v_proj(12),
                             lambda: out_proj(1, 1)],
                    (2, 2): [lambda: qk_proj(0, 3), lambda: v_proj(14),
                             lambda: out_proj(1, 2), lambda: out_proj(1, 3)],
                    (3, 0): [lambda: qk_proj(1, 3), lambda: out_proj(2, 0)],
                    (3, 1): [lambda: qk_proj(2, 3), lambda: out_proj(2, 1)],
                    (3, 2): [lambda: out_proj(2, 2), lambda: out_proj(2, 3)],
                }

                def emit_S(j, m, i):
                    """S pair for chunk i of q-block j, head pair m."""
                    d = SC * i - TJ * j if i >= NCH * j else 0
                    S2 = sps.tile([128, 2, TJ], F32, tag="S")
                    for k in range(2):
                        po = k * 64
                        nc.tensor.matmul(
                            S2[:, k, d:TJ],
                            lhsT=KT[po:po + 64, m, ts(i, SC)],
                            rhs=QT[po:po + 64, m, ds(j * TJ + d, TJ - d)],
                            start=True, stop=True,
                        )
                    return S2, d

                # prologue: projections needed by the first pair
                qk_proj(0, 0)
                v_proj(0)
                v_proj(2)

                pend = []          # [(S2, d)] chunks emitted ahead
                for pseq_idx, (j, m) in enumerate(pair_seq):
                    nch = NCH * j + NCH
                    nxt = pair_seq[pseq_idx + 1] if pseq_idx + 1 < len(pair_seq) \
                        else None
                    flist = list(fillers.get((j, m), ()))

                    while len(pend) < min(2, nch):
                        pend.append(emit_S(j, m, len(pend)))

                    Opair = ops.tile([65, 2, TJ], F32, tag="O")

                    for i in range(nch):
                        S2, d = pend.pop(0)
                        P2 = pp.tile([128, 2, TJ], BF16, tag="P")
                        nc.scalar.activation(P2[:, 0:2, d:TJ], S2[:, 0:2, d:TJ],
                                             AF.Exp, scale=SCALE)
                        if i >= NCH * j:
                            # causal fringe: zero the sub-diagonal of the
                            # 128-wide diag window, post-exp
                            for k in range(2):
                                nc.vector.tensor_mul(
                                    P2[:, k, d:d + 128],
                                    P2[:, k, d:d + 128],
                                    mask01[:],
                                )
                        if i + 2 < nch:
                            pend.append(emit_S(j, m, i + 2))
                        elif nxt is not None:
                            nnch = NCH * nxt[0] + NCH
                            ii = i + 2 - nch
                            if ii < min(2, nnch):
                                # cross-pair prefetch reads regions written by
                                # this pair's fillers: flush them FIRST so the
                                # write precedes the read in program order
                                while flist:
                                    flist.pop(0)()
                                pend.append(emit_S(nxt[0], nxt[1], ii))
                        for k in range(2):
                            nc.tensor.matmul(
                                Opair[:, k, d:TJ],
                                lhsT=Vt[:, i, 2 * m + k, :],
                                rhs=P2[:, k, d:TJ],
                                start=(i == 0), stop=(i == nch - 1),
                            )
                        if i % 2 == 1 and flist:
                            flist.pop(0)()
                    while flist:
                        flist.pop(0)()

                    # normalization (both heads fused): stage denominator rows
                    # to SBUF, approx-reciprocal, broadcast to 64 partitions,
                    # multiply into the PSUM->SBUF attT copy
                    sden = rp.tile([1, 2, TJ], F32, tag="r")
                    nc.vector.tensor_copy(sden[:], Opair[64:65, 0:2, :])
                    rden = rp.tile([1, 2, TJ], F32, tag="r")
                    nc.vector.reciprocal_approx_fast(rden[:], sden[:])
                    rbc = rbp.tile([64, 2, TJ], F32, tag="rb")
                    nc.gpsimd.partition_broadcast(rbc[:], rden[:])
                    nc.vector.tensor_mul(
                        attT2[:, 2 * m:2 * m + 2, ts(j, TJ)],
                        Opair[0:64, 0:2, :],
                        rbc[:],
                    )

                # tail: last block's output projection
                for q in range(NCH):
                    out_proj(NJ - 1, q)

    nc.compile()
    return nc


def _prep_inputs(x, Wq, Wk, Wv, Wp, bp):
    """Host-side shard + layout prep. Returns per-core input maps."""
    bf = ml_dtypes.bfloat16
    x = np.asarray(x, dtype=np.float32)

    def pack_w(W):  # [H, C, Dh] -> [128, NCI, H*Dh]
        Whd = np.transpose(np.asarray(W, np.float32), (1, 0, 2)).reshape(C, H * DH)
        return np.ascontiguousarray(
            Whd.reshape(NCI, 128, H * DH).transpose(1, 0, 2)
        ).astype(bf)

    wq_p, wk_p, wv_p = pack_w(Wq), pack_w(Wk), pack_w(Wv)
    # Wp rows indexed by hd = h*64+d -> wp2[d, h, e]
    wp2_p = np.ascontiguousarray(
        np.asarray(Wp, np.float32).reshape(H, DH, C).transpose(1, 0, 2)
    ).astype(bf)

    biasb = np.broadcast_to(np.asarray(bp, np.float32), (128, C)).copy()
    p = np.arange(128)[:, None]
    f = np.arange(128)[None, :]
    mask01_np = (f >= p).astype(ml_dtypes.bfloat16)

    in_maps = []
    for b in range(B):
        xT = np.ascontiguousarray(
            x[b].T.reshape(NCI, 128, T).transpose(1, 0, 2)
        ).astype(bf)
        in_maps.append({
            "xT": xT, "wq": wq_p, "wk": wk_p, "wv": wv_p, "wp2": wp2_p,
            "biasb": biasb, "mask01": mask01_np,
        })
    return in_maps


_CACHE = {}


def kernel(x, Wq, Wk, Wv, Wp, bp):
    from concourse.bass_utils import run_bass_kernel_spmd

    if "nc" not in _CACHE:
        _CACHE["nc"] = build_kernel()
    nc = _CACHE["nc"]
    in_maps = _prep_inputs(x, Wq, Wk, Wv, Wp, bp)
    res = run_bass_kernel_spmd(nc, in_maps, list(range(NCORES)))
    out = np.stack([res.results[b]["y"] for b in range(B)], axis=0)
    return out.astype(np.float32)
